# revision 37
# baseline (speedup 1.0000x reference)
"""Trainium2 Bass kernel for GQA attention (B=4, L=2048, HID=896,
14 q-heads / 2 kv-heads, HD=64, RoPE + causal mask + o_proj).

Sharding: one NeuronCore per (batch, kv-head) pair -> exactly 8 shards of
7 q-heads each. o_proj is row-sharded; partials are summed with a pairwise
ReduceScatter and the halves are concatenated on the host.

Layout strategy (all host-side prep): everything transposed (feature-major)
so attention runs as S^T = K^T-stationary matmuls, softmax denominators come
from an appended ones-column of V, and no on-device transposes are needed.
RoPE's rotate_half is folded into a second set of sign-permuted projection
weights. Matmuls run in bf16 (f32 accumulate), softmax in f32.
"""
import os
import sys

sys.path.insert(0, "/opt/trn_rl_repo")

import numpy as np
import ml_dtypes

import concourse.bass as bass
import concourse.mybir as mybir
import concourse.tile as tile
from concourse.bass_utils import run_bass_kernel_spmd

BF16NP = ml_dtypes.bfloat16
F32 = mybir.dt.float32
BF16 = mybir.dt.bfloat16

B, L, HID = 4, 2048, 896
NH, NKV, HD = 14, 2, 64
HPC = NH // NKV  # heads per core = 7
NCORES = 8
KCH = HID // 128  # 7 contraction chunks
NIB = L // 512  # 4 i-blocks
NJC = L // 128  # 16 j-chunks
NEG = -1e9
CHUNK_RS = bool(int(os.environ.get("CHUNK_RS", "1")))


def _fix_drains(nc, maxw=1):
    """This walrus build allows a single sync-wait per instruction; hoist
    excess waits onto preceding single-wait Drain instructions on the same
    engine (engine streams are in-order, so semantics are preserved)."""
    n = 0
    for fn in nc.m.functions:
        for blk in fn.blocks:
            newlist = []
            for ins in blk.instructions:
                si = getattr(ins, "sync_info", None)
                ow = list(si.on_wait) if si is not None and si.on_wait else []
                if len(ow) > maxw:
                    rest = ow[:]
                    while len(rest) > maxw:
                        chunk, rest = rest[:maxw], rest[maxw:]
                        d = mybir.InstNoOp(
                            name=f"{ins.name}-ws{n}", ins=[], outs=[]
                        )
                        d.engine = ins.engine
                        d.sync_info = mybir.SyncInfo(on_wait=chunk, on_update=[])
                        newlist.append(d)
                        n += 1
                    si.on_wait = rest
                newlist.append(ins)
            blk.instructions = newlist
    return n


def _act_manual(nc, out, in_, func, scale=1.0):
    """Emit InstActivation directly (used for Reciprocal, which the bass
    wrapper refuses; measured max rel err 1.2e-5 on TRN2 for our range)."""
    eng = nc.scalar
    ins = [
        eng.lower_ap(in_),
        mybir.ImmediateValue(dtype=F32, value=0.0),
        mybir.ImmediateValue(dtype=F32, value=scale),
        mybir.ImmediateValue(dtype=F32, value=0.0),
    ]
    return eng.add_instruction(
        mybir.InstActivation(
            name=nc.get_next_instruction_name(),
            func=func,
            ins=ins,
            outs=[eng.lower_ap(out)],
        )
    )


def build():
    nc = bass.Bass("TRN2", num_devices=NCORES, debug=False)

    xt_d = nc.dram_tensor("xt", [128, KCH, L], BF16, kind="ExternalInput")
    wq_d = nc.dram_tensor("wq", [128, KCH, 448], BF16, kind="ExternalInput")
    wq2_d = nc.dram_tensor("wq2", [128, KCH, 448], BF16, kind="ExternalInput")
    wqb_d = nc.dram_tensor("wqb", [1, 448], BF16, kind="ExternalInput")
    wq2b_d = nc.dram_tensor("wq2b", [1, 448], BF16, kind="ExternalInput")
    wk_d = nc.dram_tensor("wk", [128, KCH, 128], BF16, kind="ExternalInput")
    wk2_d = nc.dram_tensor("wk2", [128, KCH, 128], BF16, kind="ExternalInput")
    wkb_d = nc.dram_tensor("wkb", [1, 128], BF16, kind="ExternalInput")
    wk2b_d = nc.dram_tensor("wk2b", [1, 128], BF16, kind="ExternalInput")
    wv_d = nc.dram_tensor("wv", [128, KCH, 64], BF16, kind="ExternalInput")
    wvb_d = nc.dram_tensor("wvb", [1, 64], BF16, kind="ExternalInput")
    wo_d = nc.dram_tensor("wo", [128, 4, HID], BF16, kind="ExternalInput")
    cos_d = nc.dram_tensor("cos", [128, L], F32, kind="ExternalInput")
    sin_d = nc.dram_tensor("sin", [128, L], F32, kind="ExternalInput")
    mask_d = nc.dram_tensor("mask", [128, 128], F32, kind="ExternalInput")
    out_d = nc.dram_tensor("out", [L // 2, HID], BF16, kind="ExternalOutput")

    EXP = mybir.ActivationFunctionType.Exp

    with tile.TileContext(nc) as tc:
        with (
            tc.tile_pool(name="const", bufs=1) as cpool,
            tc.tile_pool(name="qt", bufs=4) as qtpool,
            tc.tile_pool(name="per", bufs=1) as perpool,
            tc.tile_pool(name="ot", bufs=7) as otpool,
            tc.tile_pool(name="wk1", bufs=3) as wk1,
            tc.tile_pool(name="wk2p", bufs=3) as wk2p,
            tc.tile_pool(name="ptp", bufs=3) as ptp,
            tc.tile_pool(name="nrm", bufs=2) as nrm,
            tc.tile_pool(name="osb", bufs=3) as osbp,
            tc.tile_pool(name="ps_sp", bufs=2, space="PSUM") as ps_sp,
            tc.tile_pool(name="ps_o", bufs=3, space="PSUM") as ps_o,
            tc.tile_pool(name="ps_rb", bufs=1, space="PSUM") as ps_rb,
            tc.tile_pool(name="dram", bufs=1, space="DRAM") as drpool,
        ):
            # ---- constants / inputs to SBUF ----
            xt = cpool.tile([128, KCH, L], BF16, tag="xt")
            for k in range(KCH):
                nc.sync.dma_start(xt[:, k, :], xt_d.ap()[:, k, :])
            wq = cpool.tile([128, KCH, 448], BF16, tag="wq")
            nc.sync.dma_start(wq[:], wq_d.ap())
            wq2 = cpool.tile([128, KCH, 448], BF16, tag="wq2")
            nc.sync.dma_start(wq2[:], wq2_d.ap())
            wk = cpool.tile([128, KCH, 128], BF16, tag="wk")
            nc.sync.dma_start(wk[:], wk_d.ap())
            wk2 = cpool.tile([128, KCH, 128], BF16, tag="wk2")
            nc.sync.dma_start(wk2[:], wk2_d.ap())
            wv = cpool.tile([128, KCH, 64], BF16, tag="wv")
            nc.sync.dma_start(wv[:], wv_d.ap())
            wo = cpool.tile([128, 4, HID], BF16, tag="wo")
            nc.sync.dma_start(wo[:], wo_d.ap())
            wqb = cpool.tile([1, 448], BF16, tag="wqb")
            nc.sync.dma_start(wqb[:], wqb_d.ap())
            wq2b = cpool.tile([1, 448], BF16, tag="wq2b")
            nc.sync.dma_start(wq2b[:], wq2b_d.ap())
            wkb = cpool.tile([1, 128], BF16, tag="wkb")
            nc.sync.dma_start(wkb[:], wkb_d.ap())
            wk2b = cpool.tile([1, 128], BF16, tag="wk2b")
            nc.sync.dma_start(wk2b[:], wk2b_d.ap())
            wvb = cpool.tile([1, 64], BF16, tag="wvb")
            nc.sync.dma_start(wvb[:], wvb_d.ap())
            cos = cpool.tile([128, L], F32, tag="cos")
            nc.sync.dma_start(cos[:], cos_d.ap())
            sin = cpool.tile([128, L], F32, tag="sin")
            nc.sync.dma_start(sin[:], sin_d.ap())
            msk = cpool.tile([128, 128], F32, tag="msk")
            nc.sync.dma_start(msk[:], mask_d.ap())
            ones_row = cpool.tile([1, L], BF16, tag="ones_row")
            nc.vector.memset(ones_row[:], 1.0)
            ones65 = cpool.tile([1, 64], BF16, tag="ones65")
            nc.vector.memset(ones65[:], 1.0)

            if CHUNK_RS:
                partials = [
                    drpool.tile([512, HID], BF16, tag=f"partial{k}",
                                name=f"partial{k}")
                    for k in range(4)
                ]
                shards = [
                    drpool.tile([256, HID], BF16, tag=f"shard{k}",
                                name=f"shard{k}")
                    for k in range(4)
                ]
            else:
                partial = drpool.tile([L, HID], BF16, tag="partial")
                shard = drpool.tile([L // 2, HID], BF16, tag="shard")

            # ---- K^T projection + RoPE (duplicated across partitions) ----
            kt = perpool.tile([128, L], BF16, tag="kt")
            for m in range(4):
                ms = bass.ts(m, 512)
                kp = ps_sp.tile([128, 512], F32, tag="sp")
                k2p = ps_sp.tile([128, 512], F32, tag="sp")
                for k in range(KCH):
                    nc.tensor.matmul(kp[:, :], wk[:, k, :], xt[:, k, ms],
                                     start=(k == 0), stop=False)
                nc.tensor.matmul(kp[:, :], wkb[0:1, :], ones_row[0:1, ms],
                                 start=False, stop=True)
                for k in range(KCH):
                    nc.tensor.matmul(k2p[:, :], wk2[:, k, :], xt[:, k, ms],
                                     start=(k == 0), stop=False)
                nc.tensor.matmul(k2p[:, :], wk2b[0:1, :], ones_row[0:1, ms],
                                 start=False, stop=True)
                t1 = wk1.tile([128, 512], F32, tag="t1")
                nc.vector.tensor_mul(t1[:, :], kp[:, :], cos[:, ms])
                t2 = wk2p.tile([128, 512], F32, tag="t2")
                nc.vector.tensor_mul(t2[:, :], k2p[:, :], sin[:, ms])
                nc.gpsimd.tensor_add(kt[:, ms], t1[:, :], t2[:, :])

            # ---- V projection (natural layout + ones column) ----
            vt = perpool.tile([128, NJC, 65], BF16, tag="vt")
            nc.vector.memset(vt[:, :, 64:65], 1.0)
            for mt in range(NJC):
                vp = ps_o.tile([128, 64], F32, tag="o")
                for k in range(KCH):
                    nc.tensor.matmul(vp[:, :], xt[:, k, bass.ts(mt, 128)],
                                     wv[:, k, :], start=(k == 0), stop=False)
                nc.tensor.matmul(vp[:, :], ones_row[0:1, bass.ts(mt, 128)],
                                 wvb[0:1, :], start=False, stop=True)
                nc.scalar.copy(vt[:, mt, 0:64], vp[:, :])

            # ---- Q^T projections + RoPE (head pairs on 128 partitions) ----
            qts = []
            for p in range(4):
                P = 128 if p < 3 else 64
                ns = bass.ds(128 * p, P)
                qt = qtpool.tile([128, L], BF16, tag="qt")
                qts.append(qt)
                for m in range(4):
                    ms = bass.ts(m, 512)
                    qp = ps_sp.tile([128, 512], F32, tag="sp")
                    q2p = ps_sp.tile([128, 512], F32, tag="sp")
                    for k in range(KCH):
                        nc.tensor.matmul(qp[0:P, :], wq[:, k, ns], xt[:, k, ms],
                                         start=(k == 0), stop=False)
                    nc.tensor.matmul(qp[0:P, :], wqb[0:1, ns], ones_row[0:1, ms],
                                     start=False, stop=True)
                    for k in range(KCH):
                        nc.tensor.matmul(q2p[0:P, :], wq2[:, k, ns], xt[:, k, ms],
                                         start=(k == 0), stop=False)
                    nc.tensor.matmul(q2p[0:P, :], wq2b[0:1, ns], ones_row[0:1, ms],
                                     start=False, stop=True)
                    t1 = wk1.tile([128, 512], F32, tag="t1")
                    nc.vector.tensor_mul(t1[0:P, :], qp[0:P, :], cos[0:P, ms])
                    t2 = wk2p.tile([128, 512], F32, tag="t2")
                    nc.vector.tensor_mul(t2[0:P, :], q2p[0:P, :], sin[0:P, ms])
                    nc.gpsimd.tensor_add(qt[0:P, ms], t1[0:P, :], t2[0:P, :])

            # ---- attention, head pairs packed on PE rows 0:64 / 64:128 ----
            # ib-major so each 512-row band of O^T completes early and its
            # o_proj + ReduceScatter chunk overlaps the next band's attention
            otp = [
                otpool.tile([128, L], BF16, tag="ot", name=f"otp{i}")
                for i in range(4)
            ]
            for ib in range(NIB):
                i0 = 512 * ib
                band_norms = []
                for p in range(4):
                    qt = qts[p]
                    has_b = p < 3
                    oa = ps_o.tile([65, 512], F32, tag="o")
                    ob = (
                        ps_o.tile([65, 512], F32, tag="o", name="ob")
                        if has_b
                        else None
                    )
                    njc = 4 * ib + 4
                    for jc in range(njc):
                        t = jc - 4 * ib  # >=0 on the diagonal blocks
                        c0 = 128 * t if t >= 0 else 0
                        cw = 512 - c0
                        cs = bass.ds(c0, cw)
                        isl = bass.ds(i0 + c0, cw)
                        jsl = bass.ts(jc, 128)
                        sp = ps_sp.tile([128, 1024], F32, tag="sp")
                        nc.tensor.matmul(sp[:, 0:512][:, cs], kt[0:64, jsl],
                                         qt[0:64, isl], start=True, stop=True)
                        if has_b:
                            nc.tensor.matmul(sp[:, 512:1024][:, cs],
                                             kt[64:128, jsl], qt[64:128, isl],
                                             start=True, stop=True)
                        if t >= 0:
                            dcs = bass.ds(c0, 128)
                            nc.vector.tensor_add(sp[:, 0:512][:, dcs],
                                                 sp[:, 0:512][:, dcs], msk[:, :])
                            if has_b:
                                nc.vector.tensor_add(sp[:, 512:1024][:, dcs],
                                                     sp[:, 512:1024][:, dcs],
                                                     msk[:, :])
                        pt = ptp.tile([128, 1024], BF16, tag="pt")
                        if has_b and t < 0:
                            nc.scalar.activation(pt[:, :], sp[:, :], EXP,
                                                 scale=0.125)
                        elif has_b:
                            sp3 = sp.rearrange("p (s c) -> p s c", s=2)
                            pt3 = pt.rearrange("p (s c) -> p s c", s=2)
                            nc.scalar.activation(pt3[:, :, c0:512],
                                                 sp3[:, :, c0:512], EXP,
                                                 scale=0.125)
                        else:
                            nc.scalar.activation(pt[:, cs], sp[:, 0:512][:, cs],
                                                 EXP, scale=0.125)
                        nc.tensor.matmul(oa[:, cs], vt[:, jc, :], pt[:, cs],
                                         start=(jc == 0), stop=(jc == njc - 1))
                        if has_b:
                            nc.tensor.matmul(ob[:, cs], vt[:, jc, :],
                                             pt[:, 512:1024][:, cs],
                                             start=(jc == 0), stop=(jc == njc - 1))
                    # evacuate O^T + its ones-column sums from PSUM early so
                    # the o-banks free up for the next pair's accumulation
                    for side, op_ in (("a", oa), ("b", ob)):
                        if op_ is None:
                            continue
                        osb65 = nrm.tile([65, 512], BF16, tag="osb65",
                                         name=f"osb65_{p}{side}")
                        nc.vector.tensor_copy(osb65[:, :], op_[0:65, :])
                        band_norms.append((p, side, osb65))

                # normalize the whole band (batches the ACT Reciprocal table
                # loads: 2 switches per band instead of per-head thrash)
                for p, side, osb65 in band_norms:
                    rec = nrm.tile([1, 512], BF16, tag="rec")
                    _act_manual(nc, rec[0:1, :], osb65[64:65, :],
                                mybir.ActivationFunctionType.Reciprocal)
                    rb = ps_rb.tile([64, 512], F32, tag="rb")
                    nc.tensor.matmul(rb[:, :], ones65[0:1, :],
                                     rec[0:1, :], start=True, stop=True)
                    rbs = nrm.tile([64, 512], BF16, tag="rbs")
                    nc.scalar.copy(rbs[:, :], rb[:, :])
                    rows = bass.ds(0, 64) if side == "a" else bass.ds(64, 64)
                    nc.vector.tensor_mul(otp[p][rows, bass.ts(ib, 512)],
                                         osb65[0:64, :], rbs[:, :])

                # ---- o_proj band (row-sharded, head pairs packed K=128) +
                # ---- this band's pairwise ReduceScatter chunk ----
                kc = ib
                for mt in range(4 * kc, 4 * kc + 4):
                    msl = bass.ts(mt, 128)
                    for ch in range(2):
                        csl = bass.ts(ch, 448)
                        op_ = ps_sp.tile([128, 448], F32, tag="sp")
                        for p in range(4):
                            P = 128 if p < 3 else 64
                            nc.tensor.matmul(
                                op_[:, :], otp[p][0:P, msl],
                                wo[0:P, p, csl],
                                start=(p == 0), stop=(p == 3),
                            )
                        osb = osbp.tile([128, 448], BF16, tag="osb")
                        nc.scalar.copy(osb[:, :], op_[:, :])
                        if CHUNK_RS:
                            nc.sync.dma_start(
                                partials[kc][bass.ts(mt - 4 * kc, 128), csl],
                                osb[:, :],
                            )
                        else:
                            nc.sync.dma_start(partial[msl, csl], osb[:, :])
                if CHUNK_RS:
                    ssl = bass.ts(kc, 256)
                    nc.gpsimd.collective_compute(
                        "ReduceScatter",
                        mybir.AluOpType.add,
                        ins=[partials[kc].opt()],
                        outs=[shards[kc].opt()],
                        replica_groups=[[0, 1], [2, 3], [4, 5], [6, 7]],
                    )
                    nc.sync.dma_start(out_d.ap()[ssl, :], shards[kc][:, :])
            if not CHUNK_RS:
                nc.gpsimd.collective_compute(
                    "ReduceScatter",
                    mybir.AluOpType.add,
                    ins=[partial.opt()],
                    outs=[shard.opt()],
                    replica_groups=[[0, 1], [2, 3], [4, 5], [6, 7]],
                )
                nc.sync.dma_start(out_d.ap(), shard[:])

    _fix_drains(nc)
    return nc


def _rot64(w):
    """rotate_half folded into weight rows, per 64-row head block."""
    out = np.empty_like(w)
    for h0 in range(0, w.shape[0], 64):
        blk = w[h0 : h0 + 64]
        out[h0 : h0 + 32] = -blk[32:64]
        out[h0 + 32 : h0 + 64] = blk[0:32]
    return out


def _kpack(wT):
    """[896, N] f32 -> [128, 7, N] bf16 contiguous (k-chunked)."""
    n = wT.shape[1]
    return np.ascontiguousarray(
        wT.reshape(KCH, 128, n).transpose(1, 0, 2).astype(BF16NP)
    )


def _wopack(wo_s):
    """wo shard [896, 448] -> [128, 4, 896] bf16: per head-pair p,
    partitions hold that pair's 128 rows of woT (= wo_s.T)."""
    woT = wo_s.T  # [448, 896]
    out = np.zeros((128, 4, HID), dtype=BF16NP)
    for p in range(4):
        rows = woT[128 * p : min(128 * p + 128, 448)]
        out[: rows.shape[0], p, :] = rows.astype(BF16NP)
    return out


_CACHE = {}


def kernel(**inputs):
    x = np.asarray(inputs["x"], dtype=np.float32)
    cos = np.asarray(inputs["cos"], dtype=np.float32)
    sin = np.asarray(inputs["sin"], dtype=np.float32)
    mask = np.asarray(inputs["mask"], dtype=np.float32)
    wq = np.asarray(inputs["wq"], dtype=np.float32)
    bq = np.asarray(inputs["bq"], dtype=np.float32)
    wk = np.asarray(inputs["wk"], dtype=np.float32)
    bk = np.asarray(inputs["bk"], dtype=np.float32)
    wv = np.asarray(inputs["wv"], dtype=np.float32)
    bv = np.asarray(inputs["bv"], dtype=np.float32)
    wo = np.asarray(inputs["wo"], dtype=np.float32)

    cosT = np.ascontiguousarray(np.tile(cos[0, 0].T, (2, 1)))  # [128, L] f32
    sinT = np.ascontiguousarray(np.tile(sin[0, 0].T, (2, 1)))
    mask_diag = np.ascontiguousarray(mask[0, 0, :128, :128].T)

    in_maps = []
    for core in range(NCORES):
        b, g = divmod(core, NKV)
        wq_s = wq[448 * g : 448 * (g + 1)]
        bq_s = bq[448 * g : 448 * (g + 1)]
        wk_s = wk[64 * g : 64 * (g + 1)]
        bk_s = bk[64 * g : 64 * (g + 1)]
        wv_s = wv[64 * g : 64 * (g + 1)]
        bv_s = bv[64 * g : 64 * (g + 1)]
        wo_s = wo[:, 448 * g : 448 * (g + 1)]  # [896, 448]
        wk_dup = np.concatenate([wk_s, wk_s], axis=0)  # [128, 896]
        bk_dup = np.concatenate([bk_s, bk_s], axis=0)
        in_maps.append({
            "xt": _kpack(x[b].T),
            "wq": _kpack(wq_s.T),
            "wq2": _kpack(_rot64(wq_s).T),
            "wqb": bq_s.astype(BF16NP)[None, :],
            "wq2b": _rot64(bq_s[:, None])[:, 0].astype(BF16NP)[None, :],
            "wk": _kpack(wk_dup.T),
            "wk2": _kpack(_rot64(wk_dup).T),
            "wkb": bk_dup.astype(BF16NP)[None, :],
            "wk2b": _rot64(bk_dup[:, None])[:, 0].astype(BF16NP)[None, :],
            "wv": _kpack(wv_s.T),
            "wvb": bv_s.astype(BF16NP)[None, :],
            "wo": _wopack(wo_s),
            "cos": cosT,
            "sin": sinT,
            "mask": mask_diag,
        })

    if "nc" not in _CACHE:
        _CACHE["nc"] = build()
    trace = bool(os.environ.get("KERNEL_TRACE"))
    res = run_bass_kernel_spmd(
        _CACHE["nc"], in_maps, core_ids=list(range(NCORES)), trace=trace
    )
    global LAST_EXEC_NS
    LAST_EXEC_NS = res.exec_time_ns
    out = np.empty((B, L, HID), dtype=np.float32)
    for b in range(B):
        lo = res.results[2 * b]["out"].astype(np.float32)
        hi = res.results[2 * b + 1]["out"].astype(np.float32)
        if CHUNK_RS:
            for kc in range(4):
                out[b, 512 * kc : 512 * kc + 256] = lo[256 * kc : 256 * kc + 256]
                out[b, 512 * kc + 256 : 512 * kc + 512] = hi[
                    256 * kc : 256 * kc + 256
                ]
        else:
            out[b, : L // 2] = lo
            out[b, L // 2 :] = hi
    return out


LAST_EXEC_NS = None


# revision 38
# speedup vs baseline: 1.0430x; 1.0430x over previous
"""Trainium2 Bass kernel for GQA attention (B=4, L=2048, HID=896,
14 q-heads / 2 kv-heads, HD=64, RoPE + causal mask + o_proj).

Sharding: one NeuronCore per (batch, kv-head) pair -> exactly 8 shards of
7 q-heads each. o_proj is row-sharded; partials are summed with a pairwise
ReduceScatter and the halves are concatenated on the host.

Layout strategy (all host-side prep): everything transposed (feature-major)
so attention runs as S^T = K^T-stationary matmuls, softmax denominators come
from an appended ones-column of V, and no on-device transposes are needed.
RoPE's rotate_half is folded into a second set of sign-permuted projection
weights. Matmuls run in bf16 (f32 accumulate), softmax in f32.
"""
import os
import sys

sys.path.insert(0, "/opt/trn_rl_repo")

import numpy as np
import ml_dtypes

import concourse.bass as bass
import concourse.mybir as mybir
import concourse.tile as tile
from concourse.bass_utils import run_bass_kernel_spmd

BF16NP = ml_dtypes.bfloat16
F32 = mybir.dt.float32
BF16 = mybir.dt.bfloat16

B, L, HID = 4, 2048, 896
NH, NKV, HD = 14, 2, 64
HPC = NH // NKV  # heads per core = 7
NCORES = 8
KCH = HID // 128  # 7 contraction chunks
NIB = L // 512  # 4 i-blocks
NJC = L // 128  # 16 j-chunks
NEG = -1e9
CHUNK_RS = bool(int(os.environ.get("CHUNK_RS", "1")))


def _fix_drains(nc, maxw=1):
    """This walrus build allows a single sync-wait per instruction; hoist
    excess waits onto preceding single-wait Drain instructions on the same
    engine (engine streams are in-order, so semantics are preserved)."""
    n = 0
    for fn in nc.m.functions:
        for blk in fn.blocks:
            newlist = []
            for ins in blk.instructions:
                si = getattr(ins, "sync_info", None)
                ow = list(si.on_wait) if si is not None and si.on_wait else []
                if len(ow) > maxw:
                    rest = ow[:]
                    while len(rest) > maxw:
                        chunk, rest = rest[:maxw], rest[maxw:]
                        d = mybir.InstNoOp(
                            name=f"{ins.name}-ws{n}", ins=[], outs=[]
                        )
                        d.engine = ins.engine
                        d.sync_info = mybir.SyncInfo(on_wait=chunk, on_update=[])
                        newlist.append(d)
                        n += 1
                    si.on_wait = rest
                newlist.append(ins)
            blk.instructions = newlist
    return n


def _act_manual(nc, out, in_, func, scale=1.0):
    """Emit InstActivation directly (used for Reciprocal, which the bass
    wrapper refuses; measured max rel err 1.2e-5 on TRN2 for our range)."""
    eng = nc.scalar
    ins = [
        eng.lower_ap(in_),
        mybir.ImmediateValue(dtype=F32, value=0.0),
        mybir.ImmediateValue(dtype=F32, value=scale),
        mybir.ImmediateValue(dtype=F32, value=0.0),
    ]
    return eng.add_instruction(
        mybir.InstActivation(
            name=nc.get_next_instruction_name(),
            func=func,
            ins=ins,
            outs=[eng.lower_ap(out)],
        )
    )


def build():
    nc = bass.Bass("TRN2", num_devices=NCORES, debug=False)

    xt_d = nc.dram_tensor("xt", [128, KCH, L], BF16, kind="ExternalInput")
    wq_d = nc.dram_tensor("wq", [128, KCH, 448], BF16, kind="ExternalInput")
    wq2_d = nc.dram_tensor("wq2", [128, KCH, 448], BF16, kind="ExternalInput")
    wqb_d = nc.dram_tensor("wqb", [1, 448], BF16, kind="ExternalInput")
    wq2b_d = nc.dram_tensor("wq2b", [1, 448], BF16, kind="ExternalInput")
    wk_d = nc.dram_tensor("wk", [128, KCH, 128], BF16, kind="ExternalInput")
    wk2_d = nc.dram_tensor("wk2", [128, KCH, 128], BF16, kind="ExternalInput")
    wkb_d = nc.dram_tensor("wkb", [1, 128], BF16, kind="ExternalInput")
    wk2b_d = nc.dram_tensor("wk2b", [1, 128], BF16, kind="ExternalInput")
    wv_d = nc.dram_tensor("wv", [128, KCH, 64], BF16, kind="ExternalInput")
    wvb_d = nc.dram_tensor("wvb", [1, 64], BF16, kind="ExternalInput")
    wo_d = nc.dram_tensor("wo", [128, 4, HID], BF16, kind="ExternalInput")
    cos_d = nc.dram_tensor("cos", [128, L], F32, kind="ExternalInput")
    sin_d = nc.dram_tensor("sin", [128, L], F32, kind="ExternalInput")
    mask_d = nc.dram_tensor("mask", [128, 128], F32, kind="ExternalInput")
    out_d = nc.dram_tensor("out", [L // 2, HID], BF16, kind="ExternalOutput")

    EXP = mybir.ActivationFunctionType.Exp

    with tile.TileContext(nc) as tc:
        with (
            tc.tile_pool(name="const", bufs=1) as cpool,
            tc.tile_pool(name="qt", bufs=4) as qtpool,
            tc.tile_pool(name="per", bufs=1) as perpool,
            tc.tile_pool(name="ot", bufs=7) as otpool,
            tc.tile_pool(name="wk1", bufs=3) as wk1,
            tc.tile_pool(name="wk2p", bufs=3) as wk2p,
            tc.tile_pool(name="ptp", bufs=3) as ptp,
            tc.tile_pool(name="nrm", bufs=2) as nrm,
            tc.tile_pool(name="osb", bufs=3) as osbp,
            tc.tile_pool(name="ps_sp", bufs=2, space="PSUM") as ps_sp,
            tc.tile_pool(name="ps_o", bufs=3, space="PSUM") as ps_o,
            tc.tile_pool(name="ps_rb", bufs=1, space="PSUM") as ps_rb,
            tc.tile_pool(name="dram", bufs=1, space="DRAM") as drpool,
        ):
            # ---- constants / inputs to SBUF ----
            xt = cpool.tile([128, KCH, L], BF16, tag="xt")
            for k in range(KCH):
                nc.sync.dma_start(xt[:, k, :], xt_d.ap()[:, k, :])
            wq = cpool.tile([128, KCH, 448], BF16, tag="wq")
            nc.sync.dma_start(wq[:], wq_d.ap())
            wq2 = cpool.tile([128, KCH, 448], BF16, tag="wq2")
            nc.sync.dma_start(wq2[:], wq2_d.ap())
            wk = cpool.tile([128, KCH, 128], BF16, tag="wk")
            nc.sync.dma_start(wk[:], wk_d.ap())
            wk2 = cpool.tile([128, KCH, 128], BF16, tag="wk2")
            nc.sync.dma_start(wk2[:], wk2_d.ap())
            wv = cpool.tile([128, KCH, 64], BF16, tag="wv")
            nc.sync.dma_start(wv[:], wv_d.ap())
            wo = cpool.tile([128, 4, HID], BF16, tag="wo")
            nc.sync.dma_start(wo[:], wo_d.ap())
            wqb = cpool.tile([1, 448], BF16, tag="wqb")
            nc.sync.dma_start(wqb[:], wqb_d.ap())
            wq2b = cpool.tile([1, 448], BF16, tag="wq2b")
            nc.sync.dma_start(wq2b[:], wq2b_d.ap())
            wkb = cpool.tile([1, 128], BF16, tag="wkb")
            nc.sync.dma_start(wkb[:], wkb_d.ap())
            wk2b = cpool.tile([1, 128], BF16, tag="wk2b")
            nc.sync.dma_start(wk2b[:], wk2b_d.ap())
            wvb = cpool.tile([1, 64], BF16, tag="wvb")
            nc.sync.dma_start(wvb[:], wvb_d.ap())
            cos = cpool.tile([128, L], F32, tag="cos")
            nc.sync.dma_start(cos[:], cos_d.ap())
            sin = cpool.tile([128, L], F32, tag="sin")
            nc.sync.dma_start(sin[:], sin_d.ap())
            msk = cpool.tile([128, 128], F32, tag="msk")
            nc.sync.dma_start(msk[:], mask_d.ap())
            ones_row = cpool.tile([1, L], BF16, tag="ones_row")
            nc.vector.memset(ones_row[:], 1.0)
            ones65 = cpool.tile([1, 64], BF16, tag="ones65")
            nc.vector.memset(ones65[:], 1.0)

            if CHUNK_RS:
                partials = [
                    drpool.tile([512, HID], BF16, tag=f"partial{k}",
                                name=f"partial{k}")
                    for k in range(4)
                ]
                shards = [
                    drpool.tile([256, HID], BF16, tag=f"shard{k}",
                                name=f"shard{k}")
                    for k in range(4)
                ]
            else:
                partial = drpool.tile([L, HID], BF16, tag="partial")
                shard = drpool.tile([L // 2, HID], BF16, tag="shard")

            # ---- K^T projection + RoPE (duplicated across partitions) ----
            kt = perpool.tile([128, L], BF16, tag="kt")
            for m in range(4):
                ms = bass.ts(m, 512)
                kp = ps_sp.tile([128, 512], F32, tag="sp")
                k2p = ps_sp.tile([128, 512], F32, tag="sp")
                for k in range(KCH):
                    nc.tensor.matmul(kp[:, :], wk[:, k, :], xt[:, k, ms],
                                     start=(k == 0), stop=False)
                nc.tensor.matmul(kp[:, :], wkb[0:1, :], ones_row[0:1, ms],
                                 start=False, stop=True)
                for k in range(KCH):
                    nc.tensor.matmul(k2p[:, :], wk2[:, k, :], xt[:, k, ms],
                                     start=(k == 0), stop=False)
                nc.tensor.matmul(k2p[:, :], wk2b[0:1, :], ones_row[0:1, ms],
                                 start=False, stop=True)
                t1 = wk1.tile([128, 512], F32, tag="t1")
                nc.vector.tensor_mul(t1[:, :], kp[:, :], cos[:, ms])
                t2 = wk2p.tile([128, 512], F32, tag="t2")
                nc.vector.tensor_mul(t2[:, :], k2p[:, :], sin[:, ms])
                nc.gpsimd.tensor_add(kt[:, ms], t1[:, :], t2[:, :])

            # ---- V projection (natural layout + ones column) ----
            vt = perpool.tile([128, NJC, 65], BF16, tag="vt")
            nc.vector.memset(vt[:, :, 64:65], 1.0)
            for mt in range(NJC):
                vp = ps_o.tile([128, 64], F32, tag="o")
                for k in range(KCH):
                    nc.tensor.matmul(vp[:, :], xt[:, k, bass.ts(mt, 128)],
                                     wv[:, k, :], start=(k == 0), stop=False)
                nc.tensor.matmul(vp[:, :], ones_row[0:1, bass.ts(mt, 128)],
                                 wvb[0:1, :], start=False, stop=True)
                nc.scalar.copy(vt[:, mt, 0:64], vp[:, :])

            # ---- Q^T projections + RoPE (head pairs on 128 partitions) ----
            qts = []
            for p in range(4):
                P = 128 if p < 3 else 64
                ns = bass.ds(128 * p, P)
                qt = qtpool.tile([128, L], BF16, tag="qt")
                qts.append(qt)
                for m in range(4):
                    ms = bass.ts(m, 512)
                    qp = ps_sp.tile([128, 512], F32, tag="sp")
                    q2p = ps_sp.tile([128, 512], F32, tag="sp")
                    for k in range(KCH):
                        nc.tensor.matmul(qp[0:P, :], wq[:, k, ns], xt[:, k, ms],
                                         start=(k == 0), stop=False)
                    nc.tensor.matmul(qp[0:P, :], wqb[0:1, ns], ones_row[0:1, ms],
                                     start=False, stop=True)
                    for k in range(KCH):
                        nc.tensor.matmul(q2p[0:P, :], wq2[:, k, ns], xt[:, k, ms],
                                         start=(k == 0), stop=False)
                    nc.tensor.matmul(q2p[0:P, :], wq2b[0:1, ns], ones_row[0:1, ms],
                                     start=False, stop=True)
                    t1 = wk1.tile([128, 512], F32, tag="t1")
                    nc.vector.tensor_mul(t1[0:P, :], qp[0:P, :], cos[0:P, ms])
                    t2 = wk2p.tile([128, 512], F32, tag="t2")
                    nc.vector.tensor_mul(t2[0:P, :], q2p[0:P, :], sin[0:P, ms])
                    nc.gpsimd.tensor_add(qt[0:P, ms], t1[0:P, :], t2[0:P, :])

            # ---- attention, head pairs packed on PE rows 0:64 / 64:128 ----
            # ib-major so each 512-row band of O^T completes early and its
            # o_proj + ReduceScatter chunk overlaps the next band's attention
            otp = [
                otpool.tile([128, L], BF16, tag="ot", name=f"otp{i}")
                for i in range(4)
            ]
            for ib in range(NIB):
                i0 = 512 * ib
                band_norms = []
                for p in range(4):
                    qt = qts[p]
                    has_b = p < 3
                    oa = ps_o.tile([65, 512], F32, tag="o")
                    ob = (
                        ps_o.tile([65, 512], F32, tag="o", name="ob")
                        if has_b
                        else None
                    )
                    njc = 4 * ib + 4
                    for jc in range(njc):
                        t = jc - 4 * ib  # >=0 on the diagonal blocks
                        c0 = 128 * t if t >= 0 else 0
                        cw = 512 - c0
                        cs = bass.ds(c0, cw)
                        isl = bass.ds(i0 + c0, cw)
                        jsl = bass.ts(jc, 128)
                        sp = ps_sp.tile([128, 1024], F32, tag="sp")
                        nc.tensor.matmul(sp[:, 0:512][:, cs], kt[0:64, jsl],
                                         qt[0:64, isl], start=True, stop=True)
                        if has_b:
                            nc.tensor.matmul(sp[:, 512:1024][:, cs],
                                             kt[64:128, jsl], qt[64:128, isl],
                                             start=True, stop=True)
                        if t >= 0:
                            dcs = bass.ds(c0, 128)
                            nc.vector.tensor_add(sp[:, 0:512][:, dcs],
                                                 sp[:, 0:512][:, dcs], msk[:, :])
                            if has_b:
                                nc.vector.tensor_add(sp[:, 512:1024][:, dcs],
                                                     sp[:, 512:1024][:, dcs],
                                                     msk[:, :])
                        pt = ptp.tile([128, 1024], BF16, tag="pt")
                        if has_b and t < 0:
                            nc.scalar.activation(pt[:, :], sp[:, :], EXP,
                                                 scale=0.125)
                        elif has_b:
                            sp3 = sp.rearrange("p (s c) -> p s c", s=2)
                            pt3 = pt.rearrange("p (s c) -> p s c", s=2)
                            nc.scalar.activation(pt3[:, :, c0:512],
                                                 sp3[:, :, c0:512], EXP,
                                                 scale=0.125)
                        else:
                            nc.scalar.activation(pt[:, cs], sp[:, 0:512][:, cs],
                                                 EXP, scale=0.125)
                        nc.tensor.matmul(oa[:, cs], vt[:, jc, :], pt[:, cs],
                                         start=(jc == 0), stop=(jc == njc - 1))
                        if has_b:
                            nc.tensor.matmul(ob[:, cs], vt[:, jc, :],
                                             pt[:, 512:1024][:, cs],
                                             start=(jc == 0), stop=(jc == njc - 1))
                    # evacuate O^T + its ones-column sums from PSUM early so
                    # the o-banks free up for the next pair's accumulation,
                    # then normalize from SBUF
                    for side, op_ in (("a", oa), ("b", ob)):
                        if op_ is None:
                            continue
                        osb65 = nrm.tile([65, 512], BF16, tag="osb65",
                                         name=f"osb65_{p}{side}")
                        nc.vector.tensor_copy(osb65[:, :], op_[0:65, :])
                        rec = nrm.tile([1, 512], BF16, tag="rec")
                        _act_manual(nc, rec[0:1, :], osb65[64:65, :],
                                    mybir.ActivationFunctionType.Reciprocal)
                        rb = ps_rb.tile([64, 512], F32, tag="rb")
                        nc.tensor.matmul(rb[:, :], ones65[0:1, :],
                                         rec[0:1, :], start=True, stop=True)
                        rbs = nrm.tile([64, 512], BF16, tag="rbs")
                        nc.scalar.copy(rbs[:, :], rb[:, :])
                        rows = (
                            bass.ds(0, 64) if side == "a" else bass.ds(64, 64)
                        )
                        nc.vector.tensor_mul(otp[p][rows, bass.ts(ib, 512)],
                                             osb65[0:64, :], rbs[:, :])

                # ---- o_proj band (row-sharded, head pairs packed K=128) +
                # ---- this band's pairwise ReduceScatter chunk ----
                kc = ib
                for mt in range(4 * kc, 4 * kc + 4):
                    msl = bass.ts(mt, 128)
                    for ch in range(2):
                        csl = bass.ts(ch, 448)
                        op_ = ps_sp.tile([128, 448], F32, tag="sp")
                        for p in range(4):
                            P = 128 if p < 3 else 64
                            nc.tensor.matmul(
                                op_[:, :], otp[p][0:P, msl],
                                wo[0:P, p, csl],
                                start=(p == 0), stop=(p == 3),
                            )
                        osb = osbp.tile([128, 448], BF16, tag="osb")
                        nc.scalar.copy(osb[:, :], op_[:, :])
                        if CHUNK_RS:
                            nc.sync.dma_start(
                                partials[kc][bass.ts(mt - 4 * kc, 128), csl],
                                osb[:, :],
                            )
                        else:
                            nc.sync.dma_start(partial[msl, csl], osb[:, :])
                if CHUNK_RS:
                    ssl = bass.ts(kc, 256)
                    nc.gpsimd.collective_compute(
                        "ReduceScatter",
                        mybir.AluOpType.add,
                        ins=[partials[kc].opt()],
                        outs=[shards[kc].opt()],
                        replica_groups=[[0, 1], [2, 3], [4, 5], [6, 7]],
                    )
                    nc.sync.dma_start(out_d.ap()[ssl, :], shards[kc][:, :])
            if not CHUNK_RS:
                nc.gpsimd.collective_compute(
                    "ReduceScatter",
                    mybir.AluOpType.add,
                    ins=[partial.opt()],
                    outs=[shard.opt()],
                    replica_groups=[[0, 1], [2, 3], [4, 5], [6, 7]],
                )
                nc.sync.dma_start(out_d.ap(), shard[:])

    _fix_drains(nc)
    return nc


def _rot64(w):
    """rotate_half folded into weight rows, per 64-row head block."""
    out = np.empty_like(w)
    for h0 in range(0, w.shape[0], 64):
        blk = w[h0 : h0 + 64]
        out[h0 : h0 + 32] = -blk[32:64]
        out[h0 + 32 : h0 + 64] = blk[0:32]
    return out


def _kpack(wT):
    """[896, N] f32 -> [128, 7, N] bf16 contiguous (k-chunked)."""
    n = wT.shape[1]
    return np.ascontiguousarray(
        wT.reshape(KCH, 128, n).transpose(1, 0, 2).astype(BF16NP)
    )


def _wopack(wo_s):
    """wo shard [896, 448] -> [128, 4, 896] bf16: per head-pair p,
    partitions hold that pair's 128 rows of woT (= wo_s.T)."""
    woT = wo_s.T  # [448, 896]
    out = np.zeros((128, 4, HID), dtype=BF16NP)
    for p in range(4):
        rows = woT[128 * p : min(128 * p + 128, 448)]
        out[: rows.shape[0], p, :] = rows.astype(BF16NP)
    return out


_CACHE = {}


def kernel(**inputs):
    x = np.asarray(inputs["x"], dtype=np.float32)
    cos = np.asarray(inputs["cos"], dtype=np.float32)
    sin = np.asarray(inputs["sin"], dtype=np.float32)
    mask = np.asarray(inputs["mask"], dtype=np.float32)
    wq = np.asarray(inputs["wq"], dtype=np.float32)
    bq = np.asarray(inputs["bq"], dtype=np.float32)
    wk = np.asarray(inputs["wk"], dtype=np.float32)
    bk = np.asarray(inputs["bk"], dtype=np.float32)
    wv = np.asarray(inputs["wv"], dtype=np.float32)
    bv = np.asarray(inputs["bv"], dtype=np.float32)
    wo = np.asarray(inputs["wo"], dtype=np.float32)

    cosT = np.ascontiguousarray(np.tile(cos[0, 0].T, (2, 1)))  # [128, L] f32
    sinT = np.ascontiguousarray(np.tile(sin[0, 0].T, (2, 1)))
    mask_diag = np.ascontiguousarray(mask[0, 0, :128, :128].T)

    in_maps = []
    for core in range(NCORES):
        b, g = divmod(core, NKV)
        wq_s = wq[448 * g : 448 * (g + 1)]
        bq_s = bq[448 * g : 448 * (g + 1)]
        wk_s = wk[64 * g : 64 * (g + 1)]
        bk_s = bk[64 * g : 64 * (g + 1)]
        wv_s = wv[64 * g : 64 * (g + 1)]
        bv_s = bv[64 * g : 64 * (g + 1)]
        wo_s = wo[:, 448 * g : 448 * (g + 1)]  # [896, 448]
        wk_dup = np.concatenate([wk_s, wk_s], axis=0)  # [128, 896]
        bk_dup = np.concatenate([bk_s, bk_s], axis=0)
        in_maps.append({
            "xt": _kpack(x[b].T),
            "wq": _kpack(wq_s.T),
            "wq2": _kpack(_rot64(wq_s).T),
            "wqb": bq_s.astype(BF16NP)[None, :],
            "wq2b": _rot64(bq_s[:, None])[:, 0].astype(BF16NP)[None, :],
            "wk": _kpack(wk_dup.T),
            "wk2": _kpack(_rot64(wk_dup).T),
            "wkb": bk_dup.astype(BF16NP)[None, :],
            "wk2b": _rot64(bk_dup[:, None])[:, 0].astype(BF16NP)[None, :],
            "wv": _kpack(wv_s.T),
            "wvb": bv_s.astype(BF16NP)[None, :],
            "wo": _wopack(wo_s),
            "cos": cosT,
            "sin": sinT,
            "mask": mask_diag,
        })

    if "nc" not in _CACHE:
        _CACHE["nc"] = build()
    trace = bool(os.environ.get("KERNEL_TRACE"))
    res = run_bass_kernel_spmd(
        _CACHE["nc"], in_maps, core_ids=list(range(NCORES)), trace=trace
    )
    global LAST_EXEC_NS
    LAST_EXEC_NS = res.exec_time_ns
    out = np.empty((B, L, HID), dtype=np.float32)
    for b in range(B):
        lo = res.results[2 * b]["out"].astype(np.float32)
        hi = res.results[2 * b + 1]["out"].astype(np.float32)
        if CHUNK_RS:
            for kc in range(4):
                out[b, 512 * kc : 512 * kc + 256] = lo[256 * kc : 256 * kc + 256]
                out[b, 512 * kc + 256 : 512 * kc + 512] = hi[
                    256 * kc : 256 * kc + 256
                ]
        else:
            out[b, : L // 2] = lo
            out[b, L // 2 :] = hi
    return out


LAST_EXEC_NS = None


# revision 39
# speedup vs baseline: 1.0791x; 1.0346x over previous
"""Trainium2 Bass kernel for GQA attention (B=4, L=2048, HID=896,
14 q-heads / 2 kv-heads, HD=64, RoPE + causal mask + o_proj).

Sharding: one NeuronCore per (batch, kv-head) pair -> exactly 8 shards of
7 q-heads each. o_proj is row-sharded; partials are summed with a pairwise
ReduceScatter and the halves are concatenated on the host.

Layout strategy (all host-side prep): everything transposed (feature-major)
so attention runs as S^T = K^T-stationary matmuls, softmax denominators come
from an appended ones-column of V, and no on-device transposes are needed.
RoPE's rotate_half is folded into a second set of sign-permuted projection
weights. Matmuls run in bf16 (f32 accumulate), softmax in f32.
"""
import os
import sys

sys.path.insert(0, "/opt/trn_rl_repo")

import numpy as np
import ml_dtypes

import concourse.bass as bass
import concourse.mybir as mybir
import concourse.tile as tile
from concourse.bass_utils import run_bass_kernel_spmd

BF16NP = ml_dtypes.bfloat16
F32 = mybir.dt.float32
BF16 = mybir.dt.bfloat16

B, L, HID = 4, 2048, 896
NH, NKV, HD = 14, 2, 64
HPC = NH // NKV  # heads per core = 7
NCORES = 8
KCH = HID // 128  # 7 contraction chunks
NIB = L // 512  # 4 i-blocks
NJC = L // 128  # 16 j-chunks
NEG = -1e9
CHUNK_RS = bool(int(os.environ.get("CHUNK_RS", "1")))


def _fix_drains(nc, maxw=1):
    """This walrus build allows a single sync-wait per instruction; hoist
    excess waits onto preceding single-wait Drain instructions on the same
    engine (engine streams are in-order, so semantics are preserved)."""
    n = 0
    for fn in nc.m.functions:
        for blk in fn.blocks:
            newlist = []
            for ins in blk.instructions:
                si = getattr(ins, "sync_info", None)
                ow = list(si.on_wait) if si is not None and si.on_wait else []
                if len(ow) > maxw:
                    rest = ow[:]
                    while len(rest) > maxw:
                        chunk, rest = rest[:maxw], rest[maxw:]
                        d = mybir.InstNoOp(
                            name=f"{ins.name}-ws{n}", ins=[], outs=[]
                        )
                        d.engine = ins.engine
                        d.sync_info = mybir.SyncInfo(on_wait=chunk, on_update=[])
                        newlist.append(d)
                        n += 1
                    si.on_wait = rest
                newlist.append(ins)
            blk.instructions = newlist
    return n


def _act_manual(nc, out, in_, func, scale=1.0):
    """Emit InstActivation directly (used for Reciprocal, which the bass
    wrapper refuses; measured max rel err 1.2e-5 on TRN2 for our range)."""
    eng = nc.scalar
    ins = [
        eng.lower_ap(in_),
        mybir.ImmediateValue(dtype=F32, value=0.0),
        mybir.ImmediateValue(dtype=F32, value=scale),
        mybir.ImmediateValue(dtype=F32, value=0.0),
    ]
    return eng.add_instruction(
        mybir.InstActivation(
            name=nc.get_next_instruction_name(),
            func=func,
            ins=ins,
            outs=[eng.lower_ap(out)],
        )
    )


def build():
    nc = bass.Bass("TRN2", num_devices=NCORES, debug=False)

    xt_d = nc.dram_tensor("xt", [128, KCH, L], BF16, kind="ExternalInput")
    wq_d = nc.dram_tensor("wq", [128, KCH, 448], BF16, kind="ExternalInput")
    wq2_d = nc.dram_tensor("wq2", [128, KCH, 448], BF16, kind="ExternalInput")
    wqb_d = nc.dram_tensor("wqb", [1, 448], BF16, kind="ExternalInput")
    wq2b_d = nc.dram_tensor("wq2b", [1, 448], BF16, kind="ExternalInput")
    wk_d = nc.dram_tensor("wk", [128, KCH, 128], BF16, kind="ExternalInput")
    wk2_d = nc.dram_tensor("wk2", [128, KCH, 128], BF16, kind="ExternalInput")
    wkb_d = nc.dram_tensor("wkb", [1, 128], BF16, kind="ExternalInput")
    wk2b_d = nc.dram_tensor("wk2b", [1, 128], BF16, kind="ExternalInput")
    wv_d = nc.dram_tensor("wv", [128, KCH, 64], BF16, kind="ExternalInput")
    wvb_d = nc.dram_tensor("wvb", [1, 64], BF16, kind="ExternalInput")
    wo_d = nc.dram_tensor("wo", [128, 4, HID], BF16, kind="ExternalInput")
    cos_d = nc.dram_tensor("cos", [128, L], F32, kind="ExternalInput")
    sin_d = nc.dram_tensor("sin", [128, L], F32, kind="ExternalInput")
    mask_d = nc.dram_tensor("mask", [128, 128], F32, kind="ExternalInput")
    out_d = nc.dram_tensor("out", [L // 2, HID], BF16, kind="ExternalOutput")

    EXP = mybir.ActivationFunctionType.Exp

    with tile.TileContext(nc) as tc:
        with (
            tc.tile_pool(name="const", bufs=1) as cpool,
            tc.tile_pool(name="qt", bufs=4) as qtpool,
            tc.tile_pool(name="per", bufs=1) as perpool,
            tc.tile_pool(name="ot", bufs=7) as otpool,
            tc.tile_pool(name="wk1", bufs=3) as wk1,
            tc.tile_pool(name="wk2p", bufs=3) as wk2p,
            tc.tile_pool(name="ptp", bufs=3) as ptp,
            tc.tile_pool(name="nrm", bufs=2) as nrm,
            tc.tile_pool(name="osb", bufs=3) as osbp,
            tc.tile_pool(name="ps_sp", bufs=2, space="PSUM") as ps_sp,
            tc.tile_pool(name="ps_o", bufs=3, space="PSUM") as ps_o,
            tc.tile_pool(name="ps_rb", bufs=1, space="PSUM") as ps_rb,
            tc.tile_pool(name="dram", bufs=1, space="DRAM") as drpool,
        ):
            # ---- constants / inputs to SBUF ----
            xt = cpool.tile([128, KCH, L], BF16, tag="xt")
            for k in range(KCH):
                nc.sync.dma_start(xt[:, k, :], xt_d.ap()[:, k, :])
            wq = cpool.tile([128, KCH, 448], BF16, tag="wq")
            nc.sync.dma_start(wq[:], wq_d.ap())
            wq2 = cpool.tile([128, KCH, 448], BF16, tag="wq2")
            nc.sync.dma_start(wq2[:], wq2_d.ap())
            wk = cpool.tile([128, KCH, 128], BF16, tag="wk")
            nc.sync.dma_start(wk[:], wk_d.ap())
            wk2 = cpool.tile([128, KCH, 128], BF16, tag="wk2")
            nc.sync.dma_start(wk2[:], wk2_d.ap())
            wv = cpool.tile([128, KCH, 64], BF16, tag="wv")
            nc.sync.dma_start(wv[:], wv_d.ap())
            wo = cpool.tile([128, 4, HID], BF16, tag="wo")
            nc.sync.dma_start(wo[:], wo_d.ap())
            wqb = cpool.tile([1, 448], BF16, tag="wqb")
            nc.sync.dma_start(wqb[:], wqb_d.ap())
            wq2b = cpool.tile([1, 448], BF16, tag="wq2b")
            nc.sync.dma_start(wq2b[:], wq2b_d.ap())
            wkb = cpool.tile([1, 128], BF16, tag="wkb")
            nc.sync.dma_start(wkb[:], wkb_d.ap())
            wk2b = cpool.tile([1, 128], BF16, tag="wk2b")
            nc.sync.dma_start(wk2b[:], wk2b_d.ap())
            wvb = cpool.tile([1, 64], BF16, tag="wvb")
            nc.sync.dma_start(wvb[:], wvb_d.ap())
            cos = cpool.tile([128, L], F32, tag="cos")
            nc.sync.dma_start(cos[:], cos_d.ap())
            sin = cpool.tile([128, L], F32, tag="sin")
            nc.sync.dma_start(sin[:], sin_d.ap())
            msk = cpool.tile([128, 128], F32, tag="msk")
            nc.sync.dma_start(msk[:], mask_d.ap())
            ones_row = cpool.tile([1, L], BF16, tag="ones_row")
            nc.vector.memset(ones_row[:], 1.0)
            ones65 = cpool.tile([1, 64], BF16, tag="ones65")
            nc.vector.memset(ones65[:], 1.0)

            if CHUNK_RS:
                partials = [
                    drpool.tile([512, HID], BF16, tag=f"partial{k}",
                                name=f"partial{k}")
                    for k in range(4)
                ]
                shards = [
                    drpool.tile([256, HID], BF16, tag=f"shard{k}",
                                name=f"shard{k}")
                    for k in range(4)
                ]
            else:
                partial = drpool.tile([L, HID], BF16, tag="partial")
                shard = drpool.tile([L // 2, HID], BF16, tag="shard")

            # ---- K^T projection + RoPE (duplicated across partitions) ----
            kt = perpool.tile([128, L], BF16, tag="kt")
            for m in range(4):
                ms = bass.ts(m, 512)
                kp = ps_sp.tile([128, 512], F32, tag="sp")
                k2p = ps_sp.tile([128, 512], F32, tag="sp")
                for k in range(KCH):
                    nc.tensor.matmul(kp[:, :], wk[:, k, :], xt[:, k, ms],
                                     start=(k == 0), stop=False)
                nc.tensor.matmul(kp[:, :], wkb[0:1, :], ones_row[0:1, ms],
                                 start=False, stop=True)
                for k in range(KCH):
                    nc.tensor.matmul(k2p[:, :], wk2[:, k, :], xt[:, k, ms],
                                     start=(k == 0), stop=False)
                nc.tensor.matmul(k2p[:, :], wk2b[0:1, :], ones_row[0:1, ms],
                                 start=False, stop=True)
                t1 = wk1.tile([128, 512], F32, tag="t1")
                nc.vector.tensor_mul(t1[:, :], kp[:, :], cos[:, ms])
                t2 = wk2p.tile([128, 512], F32, tag="t2")
                nc.vector.tensor_mul(t2[:, :], k2p[:, :], sin[:, ms])
                nc.gpsimd.tensor_add(kt[:, ms], t1[:, :], t2[:, :])

            # ---- V projection (natural layout + ones column) ----
            vt = perpool.tile([128, NJC, 65], BF16, tag="vt")
            nc.vector.memset(vt[:, :, 64:65], 1.0)
            for mt in range(NJC):
                vp = ps_o.tile([128, 64], F32, tag="o")
                for k in range(KCH):
                    nc.tensor.matmul(vp[:, :], xt[:, k, bass.ts(mt, 128)],
                                     wv[:, k, :], start=(k == 0), stop=False)
                nc.tensor.matmul(vp[:, :], ones_row[0:1, bass.ts(mt, 128)],
                                 wvb[0:1, :], start=False, stop=True)
                nc.scalar.copy(vt[:, mt, 0:64], vp[:, :])

            # ---- Q^T projections + RoPE (head pairs on 128 partitions) ----
            qts = []
            for p in range(4):
                P = 128 if p < 3 else 64
                ns = bass.ds(128 * p, P)
                qt = qtpool.tile([128, L], BF16, tag="qt")
                qts.append(qt)
                for m in range(4):
                    ms = bass.ts(m, 512)
                    qp = ps_sp.tile([128, 512], F32, tag="sp")
                    q2p = ps_sp.tile([128, 512], F32, tag="sp")
                    for k in range(KCH):
                        nc.tensor.matmul(qp[0:P, :], wq[:, k, ns], xt[:, k, ms],
                                         start=(k == 0), stop=False)
                    nc.tensor.matmul(qp[0:P, :], wqb[0:1, ns], ones_row[0:1, ms],
                                     start=False, stop=True)
                    for k in range(KCH):
                        nc.tensor.matmul(q2p[0:P, :], wq2[:, k, ns], xt[:, k, ms],
                                         start=(k == 0), stop=False)
                    nc.tensor.matmul(q2p[0:P, :], wq2b[0:1, ns], ones_row[0:1, ms],
                                     start=False, stop=True)
                    t1 = wk1.tile([128, 512], F32, tag="t1")
                    nc.vector.tensor_mul(t1[0:P, :], qp[0:P, :], cos[0:P, ms])
                    t2 = wk2p.tile([128, 512], F32, tag="t2")
                    nc.vector.tensor_mul(t2[0:P, :], q2p[0:P, :], sin[0:P, ms])
                    nc.gpsimd.tensor_add(qt[0:P, ms], t1[0:P, :], t2[0:P, :])

            # ---- attention, head pairs packed on PE rows 0:64 / 64:128 ----
            # ib-major so each 512-row band of O^T completes early and its
            # o_proj + ReduceScatter chunk overlaps the next band's attention
            otp = [
                otpool.tile([128, L], BF16, tag="ot", name=f"otp{i}")
                for i in range(4)
            ]
            for ib in range(NIB):
                i0 = 512 * ib
                band_norms = []
                for p in range(4):
                    qt = qts[p]
                    has_b = p < 3
                    oa = ps_o.tile([65, 512], F32, tag="o")
                    ob = (
                        ps_o.tile([65, 512], F32, tag="o", name="ob")
                        if has_b
                        else None
                    )
                    njc = 4 * ib + 4
                    for jc in range(njc):
                        t = jc - 4 * ib  # >=0 on the diagonal blocks
                        c0 = 128 * t if t >= 0 else 0
                        cw = 512 - c0
                        cs = bass.ds(c0, cw)
                        isl = bass.ds(i0 + c0, cw)
                        jsl = bass.ts(jc, 128)
                        sp = ps_sp.tile([128, 1024], F32, tag="sp")
                        nc.tensor.matmul(sp[:, 0:512][:, cs], kt[0:64, jsl],
                                         qt[0:64, isl], start=True, stop=True)
                        if has_b:
                            nc.tensor.matmul(sp[:, 512:1024][:, cs],
                                             kt[64:128, jsl], qt[64:128, isl],
                                             start=True, stop=True)
                        if t >= 0:
                            dcs = bass.ds(c0, 128)
                            nc.vector.tensor_add(sp[:, 0:512][:, dcs],
                                                 sp[:, 0:512][:, dcs], msk[:, :])
                            if has_b:
                                nc.vector.tensor_add(sp[:, 512:1024][:, dcs],
                                                     sp[:, 512:1024][:, dcs],
                                                     msk[:, :])
                        pt = ptp.tile([128, 1024], BF16, tag="pt")
                        if has_b and t < 0:
                            nc.scalar.activation(pt[:, :], sp[:, :], EXP,
                                                 scale=0.125)
                        elif has_b:
                            sp3 = sp.rearrange("p (s c) -> p s c", s=2)
                            pt3 = pt.rearrange("p (s c) -> p s c", s=2)
                            nc.scalar.activation(pt3[:, :, c0:512],
                                                 sp3[:, :, c0:512], EXP,
                                                 scale=0.125)
                        else:
                            nc.scalar.activation(pt[:, cs], sp[:, 0:512][:, cs],
                                                 EXP, scale=0.125)
                        nc.tensor.matmul(oa[:, cs], vt[:, jc, :], pt[:, cs],
                                         start=(jc == 0), stop=(jc == njc - 1))
                        if has_b:
                            nc.tensor.matmul(ob[:, cs], vt[:, jc, :],
                                             pt[:, 512:1024][:, cs],
                                             start=(jc == 0), stop=(jc == njc - 1))
                    # normalize: divide by the ones-column sums (row 64)
                    for side, op_ in (("a", oa), ("b", ob)):
                        if op_ is None:
                            continue
                        rec = nrm.tile([1, 512], BF16, tag="rec")
                        _act_manual(nc, rec[0:1, :], op_[64:65, :],
                                    mybir.ActivationFunctionType.Reciprocal)
                        rb = ps_rb.tile([64, 512], F32, tag="rb")
                        nc.tensor.matmul(rb[:, :], ones65[0:1, :],
                                         rec[0:1, :], start=True, stop=True)
                        rbs = nrm.tile([64, 512], BF16, tag="rbs")
                        nc.scalar.copy(rbs[:, :], rb[:, :])
                        rows = (
                            bass.ds(0, 64) if side == "a" else bass.ds(64, 64)
                        )
                        nc.vector.tensor_mul(otp[p][rows, bass.ts(ib, 512)],
                                             op_[0:64, :], rbs[:, :])

                # ---- o_proj band (row-sharded, head pairs packed K=128) +
                # ---- this band's pairwise ReduceScatter chunk ----
                kc = ib
                for mt in range(4 * kc, 4 * kc + 4):
                    msl = bass.ts(mt, 128)
                    for ch in range(2):
                        csl = bass.ts(ch, 448)
                        op_ = ps_sp.tile([128, 448], F32, tag="sp")
                        for p in range(4):
                            P = 128 if p < 3 else 64
                            nc.tensor.matmul(
                                op_[:, :], otp[p][0:P, msl],
                                wo[0:P, p, csl],
                                start=(p == 0), stop=(p == 3),
                            )
                        osb = osbp.tile([128, 448], BF16, tag="osb")
                        nc.scalar.copy(osb[:, :], op_[:, :])
                        if CHUNK_RS:
                            nc.sync.dma_start(
                                partials[kc][bass.ts(mt - 4 * kc, 128), csl],
                                osb[:, :],
                            )
                        else:
                            nc.sync.dma_start(partial[msl, csl], osb[:, :])
                if CHUNK_RS:
                    ssl = bass.ts(kc, 256)
                    nc.gpsimd.collective_compute(
                        "ReduceScatter",
                        mybir.AluOpType.add,
                        ins=[partials[kc].opt()],
                        outs=[shards[kc].opt()],
                        replica_groups=[[0, 1], [2, 3], [4, 5], [6, 7]],
                    )
                    nc.sync.dma_start(out_d.ap()[ssl, :], shards[kc][:, :])
            if not CHUNK_RS:
                nc.gpsimd.collective_compute(
                    "ReduceScatter",
                    mybir.AluOpType.add,
                    ins=[partial.opt()],
                    outs=[shard.opt()],
                    replica_groups=[[0, 1], [2, 3], [4, 5], [6, 7]],
                )
                nc.sync.dma_start(out_d.ap(), shard[:])

    _fix_drains(nc)
    return nc


def _rot64(w):
    """rotate_half folded into weight rows, per 64-row head block."""
    out = np.empty_like(w)
    for h0 in range(0, w.shape[0], 64):
        blk = w[h0 : h0 + 64]
        out[h0 : h0 + 32] = -blk[32:64]
        out[h0 + 32 : h0 + 64] = blk[0:32]
    return out


def _kpack(wT):
    """[896, N] f32 -> [128, 7, N] bf16 contiguous (k-chunked)."""
    n = wT.shape[1]
    return np.ascontiguousarray(
        wT.reshape(KCH, 128, n).transpose(1, 0, 2).astype(BF16NP)
    )


def _wopack(wo_s):
    """wo shard [896, 448] -> [128, 4, 896] bf16: per head-pair p,
    partitions hold that pair's 128 rows of woT (= wo_s.T)."""
    woT = wo_s.T  # [448, 896]
    out = np.zeros((128, 4, HID), dtype=BF16NP)
    for p in range(4):
        rows = woT[128 * p : min(128 * p + 128, 448)]
        out[: rows.shape[0], p, :] = rows.astype(BF16NP)
    return out


_CACHE = {}


def kernel(**inputs):
    x = np.asarray(inputs["x"], dtype=np.float32)
    cos = np.asarray(inputs["cos"], dtype=np.float32)
    sin = np.asarray(inputs["sin"], dtype=np.float32)
    mask = np.asarray(inputs["mask"], dtype=np.float32)
    wq = np.asarray(inputs["wq"], dtype=np.float32)
    bq = np.asarray(inputs["bq"], dtype=np.float32)
    wk = np.asarray(inputs["wk"], dtype=np.float32)
    bk = np.asarray(inputs["bk"], dtype=np.float32)
    wv = np.asarray(inputs["wv"], dtype=np.float32)
    bv = np.asarray(inputs["bv"], dtype=np.float32)
    wo = np.asarray(inputs["wo"], dtype=np.float32)

    cosT = np.ascontiguousarray(np.tile(cos[0, 0].T, (2, 1)))  # [128, L] f32
    sinT = np.ascontiguousarray(np.tile(sin[0, 0].T, (2, 1)))
    mask_diag = np.ascontiguousarray(mask[0, 0, :128, :128].T)

    in_maps = []
    for core in range(NCORES):
        b, g = divmod(core, NKV)
        wq_s = wq[448 * g : 448 * (g + 1)]
        bq_s = bq[448 * g : 448 * (g + 1)]
        wk_s = wk[64 * g : 64 * (g + 1)]
        bk_s = bk[64 * g : 64 * (g + 1)]
        wv_s = wv[64 * g : 64 * (g + 1)]
        bv_s = bv[64 * g : 64 * (g + 1)]
        wo_s = wo[:, 448 * g : 448 * (g + 1)]  # [896, 448]
        wk_dup = np.concatenate([wk_s, wk_s], axis=0)  # [128, 896]
        bk_dup = np.concatenate([bk_s, bk_s], axis=0)
        in_maps.append({
            "xt": _kpack(x[b].T),
            "wq": _kpack(wq_s.T),
            "wq2": _kpack(_rot64(wq_s).T),
            "wqb": bq_s.astype(BF16NP)[None, :],
            "wq2b": _rot64(bq_s[:, None])[:, 0].astype(BF16NP)[None, :],
            "wk": _kpack(wk_dup.T),
            "wk2": _kpack(_rot64(wk_dup).T),
            "wkb": bk_dup.astype(BF16NP)[None, :],
            "wk2b": _rot64(bk_dup[:, None])[:, 0].astype(BF16NP)[None, :],
            "wv": _kpack(wv_s.T),
            "wvb": bv_s.astype(BF16NP)[None, :],
            "wo": _wopack(wo_s),
            "cos": cosT,
            "sin": sinT,
            "mask": mask_diag,
        })

    if "nc" not in _CACHE:
        _CACHE["nc"] = build()
    trace = bool(os.environ.get("KERNEL_TRACE"))
    res = run_bass_kernel_spmd(
        _CACHE["nc"], in_maps, core_ids=list(range(NCORES)), trace=trace
    )
    global LAST_EXEC_NS
    LAST_EXEC_NS = res.exec_time_ns
    out = np.empty((B, L, HID), dtype=np.float32)
    for b in range(B):
        lo = res.results[2 * b]["out"].astype(np.float32)
        hi = res.results[2 * b + 1]["out"].astype(np.float32)
        if CHUNK_RS:
            for kc in range(4):
                out[b, 512 * kc : 512 * kc + 256] = lo[256 * kc : 256 * kc + 256]
                out[b, 512 * kc + 256 : 512 * kc + 512] = hi[
                    256 * kc : 256 * kc + 256
                ]
        else:
            out[b, : L // 2] = lo
            out[b, L // 2 :] = hi
    return out


LAST_EXEC_NS = None


# revision 47
# speedup vs baseline: 1.0900x; 1.0101x over previous
"""Trainium2 Bass kernel for GQA attention (B=4, L=2048, HID=896,
14 q-heads / 2 kv-heads, HD=64, RoPE + causal mask + o_proj).

Sharding: one NeuronCore per (batch, kv-head) pair -> exactly 8 shards of
7 q-heads each. o_proj is row-sharded; partials are summed with a pairwise
ReduceScatter and the halves are concatenated on the host.

Layout strategy (all host-side prep): everything transposed (feature-major)
so attention runs as S^T = K^T-stationary matmuls, softmax denominators come
from an appended ones-column of V, and no on-device transposes are needed.
RoPE's rotate_half is folded into a second set of sign-permuted projection
weights. Matmuls run in bf16 (f32 accumulate), softmax in f32.
"""
import os
import sys

sys.path.insert(0, "/opt/trn_rl_repo")

import numpy as np
import ml_dtypes

import concourse.bass as bass
import concourse.mybir as mybir
import concourse.tile as tile
from concourse.bass_utils import run_bass_kernel_spmd

BF16NP = ml_dtypes.bfloat16
F32 = mybir.dt.float32
BF16 = mybir.dt.bfloat16

B, L, HID = 4, 2048, 896
NH, NKV, HD = 14, 2, 64
HPC = NH // NKV  # heads per core = 7
NCORES = 8
KCH = HID // 128  # 7 contraction chunks
NIB = L // 512  # 4 i-blocks
NJC = L // 128  # 16 j-chunks
NEG = -1e9
CHUNK_RS = bool(int(os.environ.get("CHUNK_RS", "1")))


def _fix_drains(nc, maxw=1):
    """This walrus build allows a single sync-wait per instruction; hoist
    excess waits onto preceding single-wait Drain instructions on the same
    engine (engine streams are in-order, so semantics are preserved)."""
    n = 0
    for fn in nc.m.functions:
        for blk in fn.blocks:
            newlist = []
            for ins in blk.instructions:
                si = getattr(ins, "sync_info", None)
                ow = list(si.on_wait) if si is not None and si.on_wait else []
                if len(ow) > maxw:
                    rest = ow[:]
                    while len(rest) > maxw:
                        chunk, rest = rest[:maxw], rest[maxw:]
                        d = mybir.InstNoOp(
                            name=f"{ins.name}-ws{n}", ins=[], outs=[]
                        )
                        d.engine = ins.engine
                        d.sync_info = mybir.SyncInfo(on_wait=chunk, on_update=[])
                        newlist.append(d)
                        n += 1
                    si.on_wait = rest
                newlist.append(ins)
            blk.instructions = newlist
    return n


def _act_manual(nc, out, in_, func, scale=1.0):
    """Emit InstActivation directly (used for Reciprocal, which the bass
    wrapper refuses; measured max rel err 1.2e-5 on TRN2 for our range)."""
    eng = nc.scalar
    ins = [
        eng.lower_ap(in_),
        mybir.ImmediateValue(dtype=F32, value=0.0),
        mybir.ImmediateValue(dtype=F32, value=scale),
        mybir.ImmediateValue(dtype=F32, value=0.0),
    ]
    return eng.add_instruction(
        mybir.InstActivation(
            name=nc.get_next_instruction_name(),
            func=func,
            ins=ins,
            outs=[eng.lower_ap(out)],
        )
    )


def build():
    nc = bass.Bass("TRN2", num_devices=NCORES, debug=False)

    xt_d = nc.dram_tensor("xt", [128, KCH, L], BF16, kind="ExternalInput")
    wq_d = nc.dram_tensor("wq", [128, KCH, 448], BF16, kind="ExternalInput")
    wqb_d = nc.dram_tensor("wqb", [1, 448], BF16, kind="ExternalInput")
    wk_d = nc.dram_tensor("wk", [128, KCH, 128], BF16, kind="ExternalInput")
    wkb_d = nc.dram_tensor("wkb", [1, 128], BF16, kind="ExternalInput")
    wv_d = nc.dram_tensor("wv", [128, KCH, 64], BF16, kind="ExternalInput")
    wvb_d = nc.dram_tensor("wvb", [1, 64], BF16, kind="ExternalInput")
    wo_d = nc.dram_tensor("wo", [128, 4, HID], BF16, kind="ExternalInput")
    cos_d = nc.dram_tensor("cos", [128, L], F32, kind="ExternalInput")
    sin_d = nc.dram_tensor("sin", [128, L], F32, kind="ExternalInput")
    mask_d = nc.dram_tensor("mask", [128, 128], F32, kind="ExternalInput")
    out_d = nc.dram_tensor("out", [L // 2, HID], BF16, kind="ExternalOutput")

    EXP = mybir.ActivationFunctionType.Exp

    with tile.TileContext(nc) as tc:
        with (
            tc.tile_pool(name="const", bufs=1) as cpool,
            tc.tile_pool(name="qt", bufs=4) as qtpool,
            tc.tile_pool(name="per", bufs=1) as perpool,
            tc.tile_pool(name="ot", bufs=7) as otpool,
            tc.tile_pool(name="wk1", bufs=3) as wk1,
            tc.tile_pool(name="wk2p", bufs=3) as wk2p,
            tc.tile_pool(name="ptp", bufs=3) as ptp,
            tc.tile_pool(name="nrm", bufs=2) as nrm,
            tc.tile_pool(name="osb", bufs=3) as osbp,
            tc.tile_pool(name="ps_sp", bufs=2, space="PSUM") as ps_sp,
            tc.tile_pool(name="ps_o", bufs=3, space="PSUM") as ps_o,
            tc.tile_pool(name="ps_rb", bufs=1, space="PSUM") as ps_rb,
            tc.tile_pool(name="dram", bufs=1, space="DRAM") as drpool,
        ):
            # ---- constants / inputs to SBUF ----
            xt = cpool.tile([128, KCH, L], BF16, tag="xt")
            for k in range(KCH):
                nc.sync.dma_start(xt[:, k, :], xt_d.ap()[:, k, :])
            wq = cpool.tile([128, KCH, 448], BF16, tag="wq")
            nc.sync.dma_start(wq[:], wq_d.ap())
            wk = cpool.tile([128, KCH, 128], BF16, tag="wk")
            nc.sync.dma_start(wk[:], wk_d.ap())
            wv = cpool.tile([128, KCH, 64], BF16, tag="wv")
            nc.sync.dma_start(wv[:], wv_d.ap())
            wo = cpool.tile([128, 4, HID], BF16, tag="wo")
            nc.sync.dma_start(wo[:], wo_d.ap())
            wqb = cpool.tile([1, 448], BF16, tag="wqb")
            nc.sync.dma_start(wqb[:], wqb_d.ap())
            wkb = cpool.tile([1, 128], BF16, tag="wkb")
            nc.sync.dma_start(wkb[:], wkb_d.ap())
            wvb = cpool.tile([1, 64], BF16, tag="wvb")
            nc.sync.dma_start(wvb[:], wvb_d.ap())
            cos = cpool.tile([128, L], F32, tag="cos")
            nc.sync.dma_start(cos[:], cos_d.ap())
            sinm = cpool.tile([128, L], F32, tag="sinm")
            nc.sync.dma_start(sinm[:], sin_d.ap())
            msk = cpool.tile([128, 128], F32, tag="msk")
            nc.sync.dma_start(msk[:], mask_d.ap())
            ones_row = cpool.tile([1, L], BF16, tag="ones_row")
            nc.vector.memset(ones_row[:], 1.0)
            ones65 = cpool.tile([1, 64], BF16, tag="ones65")
            nc.vector.memset(ones65[:], 1.0)

            if CHUNK_RS:
                partials = [
                    drpool.tile([512, HID], BF16, tag=f"partial{k}",
                                name=f"partial{k}")
                    for k in range(4)
                ]
                shards = [
                    drpool.tile([256, HID], BF16, tag=f"shard{k}",
                                name=f"shard{k}")
                    for k in range(4)
                ]
            else:
                partial = drpool.tile([L, HID], BF16, tag="partial")
                shard = drpool.tile([L // 2, HID], BF16, tag="shard")

            def rope(dst, qp, P, ms):
                """dst[:, ms] = qp*cos + rotate_half(qp)*sin, with the
                rotation done as 4 cross-partition-offset muls against the
                sign-folded sin table."""
                t1 = wk1.tile([128, 512], F32, tag="t1")
                nc.vector.tensor_mul(t1[0:P, :], qp[0:P, :], cos[0:P, ms])
                t2 = wk2p.tile([128, 512], F32, tag="t2")
                for b in range(P // 32):
                    s = 32 * (b ^ 1)
                    nc.vector.tensor_mul(
                        t2[32 * b : 32 * b + 32, :],
                        qp[s : s + 32, :],
                        sinm[32 * b : 32 * b + 32, ms],
                    )
                nc.gpsimd.tensor_add(dst[0:P, ms], t1[0:P, :], t2[0:P, :])

            # ---- K^T projection + RoPE (duplicated across partitions) ----
            kt = perpool.tile([128, L], BF16, tag="kt")
            for m in range(4):
                ms = bass.ts(m, 512)
                kp = ps_sp.tile([128, 512], F32, tag="sp")
                for k in range(KCH):
                    nc.tensor.matmul(kp[:, :], wk[:, k, :], xt[:, k, ms],
                                     start=(k == 0), stop=False)
                nc.tensor.matmul(kp[:, :], wkb[0:1, :], ones_row[0:1, ms],
                                 start=False, stop=True)
                rope(kt, kp, 128, ms)

            # ---- V projection (natural layout + ones column) ----
            vt = perpool.tile([128, NJC, 65], BF16, tag="vt")
            nc.vector.memset(vt[:, :, 64:65], 1.0)
            for mt in range(NJC):
                vp = ps_o.tile([128, 64], F32, tag="o")
                for k in range(KCH):
                    nc.tensor.matmul(vp[:, :], xt[:, k, bass.ts(mt, 128)],
                                     wv[:, k, :], start=(k == 0), stop=False)
                nc.tensor.matmul(vp[:, :], ones_row[0:1, bass.ts(mt, 128)],
                                 wvb[0:1, :], start=False, stop=True)
                nc.scalar.copy(vt[:, mt, 0:64], vp[:, :])

            # ---- Q^T projections + RoPE (head pairs on 128 partitions) ----
            qts = []
            for p in range(4):
                P = 128 if p < 3 else 64
                ns = bass.ds(128 * p, P)
                qt = qtpool.tile([128, L], BF16, tag="qt")
                qts.append(qt)
                for m in range(4):
                    ms = bass.ts(m, 512)
                    qp = ps_sp.tile([128, 512], F32, tag="sp")
                    for k in range(KCH):
                        nc.tensor.matmul(qp[0:P, :], wq[:, k, ns], xt[:, k, ms],
                                         start=(k == 0), stop=False)
                    nc.tensor.matmul(qp[0:P, :], wqb[0:1, ns], ones_row[0:1, ms],
                                     start=False, stop=True)
                    rope(qt, qp, P, ms)

            # ---- attention, head pairs packed on PE rows 0:64 / 64:128 ----
            # ib-major so each 512-row band of O^T completes early and its
            # o_proj + ReduceScatter chunk overlaps the next band's attention
            otp = [
                otpool.tile([128, L], BF16, tag="ot", name=f"otp{i}")
                for i in range(4)
            ]
            for ib in range(NIB):
                i0 = 512 * ib
                band_norms = []
                for p in range(4):
                    qt = qts[p]
                    has_b = p < 3
                    oa = ps_o.tile([65, 512], F32, tag="o")
                    ob = (
                        ps_o.tile([65, 512], F32, tag="o", name="ob")
                        if has_b
                        else None
                    )
                    njc = 4 * ib + 4
                    for jc in range(njc):
                        t = jc - 4 * ib  # >=0 on the diagonal blocks
                        c0 = 128 * t if t >= 0 else 0
                        cw = 512 - c0
                        cs = bass.ds(c0, cw)
                        isl = bass.ds(i0 + c0, cw)
                        jsl = bass.ts(jc, 128)
                        sp = ps_sp.tile([128, 1024], F32, tag="sp")
                        nc.tensor.matmul(sp[:, 0:512][:, cs], kt[0:64, jsl],
                                         qt[0:64, isl], start=True, stop=True)
                        if has_b:
                            nc.tensor.matmul(sp[:, 512:1024][:, cs],
                                             kt[64:128, jsl], qt[64:128, isl],
                                             start=True, stop=True)
                        if t >= 0:
                            dcs = bass.ds(c0, 128)
                            nc.vector.tensor_add(sp[:, 0:512][:, dcs],
                                                 sp[:, 0:512][:, dcs], msk[:, :])
                            if has_b:
                                nc.vector.tensor_add(sp[:, 512:1024][:, dcs],
                                                     sp[:, 512:1024][:, dcs],
                                                     msk[:, :])
                        pt = ptp.tile([128, 1024], BF16, tag="pt")
                        if has_b and t < 0:
                            nc.scalar.activation(pt[:, :], sp[:, :], EXP,
                                                 scale=0.125)
                        elif has_b:
                            sp3 = sp.rearrange("p (s c) -> p s c", s=2)
                            pt3 = pt.rearrange("p (s c) -> p s c", s=2)
                            nc.scalar.activation(pt3[:, :, c0:512],
                                                 sp3[:, :, c0:512], EXP,
                                                 scale=0.125)
                        else:
                            nc.scalar.activation(pt[:, cs], sp[:, 0:512][:, cs],
                                                 EXP, scale=0.125)
                        nc.tensor.matmul(oa[:, cs], vt[:, jc, :], pt[:, cs],
                                         start=(jc == 0), stop=(jc == njc - 1))
                        if has_b:
                            nc.tensor.matmul(ob[:, cs], vt[:, jc, :],
                                             pt[:, 512:1024][:, cs],
                                             start=(jc == 0), stop=(jc == njc - 1))
                    # normalize: divide by the ones-column sums (row 64)
                    for side, op_ in (("a", oa), ("b", ob)):
                        if op_ is None:
                            continue
                        rec = nrm.tile([1, 512], BF16, tag="rec")
                        _act_manual(nc, rec[0:1, :], op_[64:65, :],
                                    mybir.ActivationFunctionType.Reciprocal)
                        rb = ps_rb.tile([64, 512], F32, tag="rb")
                        nc.tensor.matmul(rb[:, :], ones65[0:1, :],
                                         rec[0:1, :], start=True, stop=True)
                        rbs = nrm.tile([64, 512], BF16, tag="rbs")
                        nc.scalar.copy(rbs[:, :], rb[:, :])
                        rows = (
                            bass.ds(0, 64) if side == "a" else bass.ds(64, 64)
                        )
                        nc.vector.tensor_mul(otp[p][rows, bass.ts(ib, 512)],
                                             op_[0:64, :], rbs[:, :])

                # ---- o_proj band (row-sharded, head pairs packed K=128) +
                # ---- this band's pairwise ReduceScatter chunk ----
                kc = ib
                for mt in range(4 * kc, 4 * kc + 4):
                    msl = bass.ts(mt, 128)
                    for ch in range(2):
                        csl = bass.ts(ch, 448)
                        op_ = ps_sp.tile([128, 448], F32, tag="sp")
                        for p in range(4):
                            P = 128 if p < 3 else 64
                            nc.tensor.matmul(
                                op_[:, :], otp[p][0:P, msl],
                                wo[0:P, p, csl],
                                start=(p == 0), stop=(p == 3),
                            )
                        osb = osbp.tile([128, 448], BF16, tag="osb")
                        nc.scalar.copy(osb[:, :], op_[:, :])
                        if CHUNK_RS:
                            nc.sync.dma_start(
                                partials[kc][bass.ts(mt - 4 * kc, 128), csl],
                                osb[:, :],
                            )
                        else:
                            nc.sync.dma_start(partial[msl, csl], osb[:, :])
                if CHUNK_RS:
                    ssl = bass.ts(kc, 256)
                    nc.gpsimd.collective_compute(
                        "ReduceScatter",
                        mybir.AluOpType.add,
                        ins=[partials[kc].opt()],
                        outs=[shards[kc].opt()],
                        replica_groups=[[0, 1], [2, 3], [4, 5], [6, 7]],
                    )
                    nc.sync.dma_start(out_d.ap()[ssl, :], shards[kc][:, :])
            if not CHUNK_RS:
                nc.gpsimd.collective_compute(
                    "ReduceScatter",
                    mybir.AluOpType.add,
                    ins=[partial.opt()],
                    outs=[shard.opt()],
                    replica_groups=[[0, 1], [2, 3], [4, 5], [6, 7]],
                )
                nc.sync.dma_start(out_d.ap(), shard[:])

    _fix_drains(nc)
    return nc


def _rot64(w):
    """rotate_half folded into weight rows, per 64-row head block."""
    out = np.empty_like(w)
    for h0 in range(0, w.shape[0], 64):
        blk = w[h0 : h0 + 64]
        out[h0 : h0 + 32] = -blk[32:64]
        out[h0 + 32 : h0 + 64] = blk[0:32]
    return out


def _kpack(wT):
    """[896, N] f32 -> [128, 7, N] bf16 contiguous (k-chunked)."""
    n = wT.shape[1]
    return np.ascontiguousarray(
        wT.reshape(KCH, 128, n).transpose(1, 0, 2).astype(BF16NP)
    )


def _wopack(wo_s):
    """wo shard [896, 448] -> [128, 4, 896] bf16: per head-pair p,
    partitions hold that pair's 128 rows of woT (= wo_s.T)."""
    woT = wo_s.T  # [448, 896]
    out = np.zeros((128, 4, HID), dtype=BF16NP)
    for p in range(4):
        rows = woT[128 * p : min(128 * p + 128, 448)]
        out[: rows.shape[0], p, :] = rows.astype(BF16NP)
    return out


_CACHE = {}


def kernel(**inputs):
    x = np.asarray(inputs["x"], dtype=np.float32)
    cos = np.asarray(inputs["cos"], dtype=np.float32)
    sin = np.asarray(inputs["sin"], dtype=np.float32)
    mask = np.asarray(inputs["mask"], dtype=np.float32)
    wq = np.asarray(inputs["wq"], dtype=np.float32)
    bq = np.asarray(inputs["bq"], dtype=np.float32)
    wk = np.asarray(inputs["wk"], dtype=np.float32)
    bk = np.asarray(inputs["bk"], dtype=np.float32)
    wv = np.asarray(inputs["wv"], dtype=np.float32)
    bv = np.asarray(inputs["bv"], dtype=np.float32)
    wo = np.asarray(inputs["wo"], dtype=np.float32)

    cosT = np.ascontiguousarray(np.tile(cos[0, 0].T, (2, 1)))  # [128, L] f32
    sinT = sin[0, 0].T  # [64, L]
    # sign-folded sin for the in-place rotate_half: out[32b:32b+32] reads
    # q[32(b^1):...] times these rows; rows 0:32 carry the minus sign
    sinm = np.ascontiguousarray(
        np.tile(np.concatenate([-sinT[0:32], sinT[32:64]], axis=0), (2, 1))
    )
    mask_diag = np.ascontiguousarray(mask[0, 0, :128, :128].T)

    in_maps = []
    for core in range(NCORES):
        b, g = divmod(core, NKV)
        wq_s = wq[448 * g : 448 * (g + 1)]
        bq_s = bq[448 * g : 448 * (g + 1)]
        wk_s = wk[64 * g : 64 * (g + 1)]
        bk_s = bk[64 * g : 64 * (g + 1)]
        wv_s = wv[64 * g : 64 * (g + 1)]
        bv_s = bv[64 * g : 64 * (g + 1)]
        wo_s = wo[:, 448 * g : 448 * (g + 1)]  # [896, 448]
        wk_dup = np.concatenate([wk_s, wk_s], axis=0)  # [128, 896]
        bk_dup = np.concatenate([bk_s, bk_s], axis=0)
        in_maps.append({
            "xt": _kpack(x[b].T),
            "wq": _kpack(wq_s.T),
            "wqb": bq_s.astype(BF16NP)[None, :],
            "wk": _kpack(wk_dup.T),
            "wkb": bk_dup.astype(BF16NP)[None, :],
            "wv": _kpack(wv_s.T),
            "wvb": bv_s.astype(BF16NP)[None, :],
            "wo": _wopack(wo_s),
            "cos": cosT,
            "sin": sinm,
            "mask": mask_diag,
        })

    if "nc" not in _CACHE:
        _CACHE["nc"] = build()
    trace = bool(os.environ.get("KERNEL_TRACE"))
    res = run_bass_kernel_spmd(
        _CACHE["nc"], in_maps, core_ids=list(range(NCORES)), trace=trace
    )
    global LAST_EXEC_NS
    LAST_EXEC_NS = res.exec_time_ns
    out = np.empty((B, L, HID), dtype=np.float32)
    for b in range(B):
        lo = res.results[2 * b]["out"].astype(np.float32)
        hi = res.results[2 * b + 1]["out"].astype(np.float32)
        if CHUNK_RS:
            for kc in range(4):
                out[b, 512 * kc : 512 * kc + 256] = lo[256 * kc : 256 * kc + 256]
                out[b, 512 * kc + 256 : 512 * kc + 512] = hi[
                    256 * kc : 256 * kc + 256
                ]
        else:
            out[b, : L // 2] = lo
            out[b, L // 2 :] = hi
    return out


LAST_EXEC_NS = None
